# revision 11
# baseline (speedup 1.0000x reference)
"""GRU/SetConv GNN message-passing kernel for 8 TRN2 NeuronCores.

Strategy (data-parallel per the sharding hint):
  core c -> batch b=c//2, point-half hh=c%2; tokens rotated so the core's own
  half is local tokens 0..4095.  Everything runs in transposed layout
  [channels, tokens]:
    1. per-token tables U_zr^T = W1zr^T @ [x;h]^T + b (PE, 16 chunks)
    2. per 128-point block: gpsimd ap_gather pulls 4096 neighbor columns
       of the SBUF-resident table (no DMA descriptors), PE adds the edge
       term ef @ W1[128:131] via 3-deep matmuls into PSUM, DVE adds and
       max-pools over k=32 with a single strided reduce.
    3. tiny MLPs run transposed with per-partition biases fused into ACT
       (Identity/Sigmoid/Tanh); LReLU via DVE mul+max.
  Gate r is computed for the full batch on both half-cores (cheaper than a
  cross-core exchange); z and q only for the own half.  The q-table reuses
  the z-table rows; z/q gate outputs and the final GRU combine reuse dead
  regions of the pooled buffer, so SBUF holds everything without spills.

HW quirks found the hard way: accumulating matmuls whose lhsT live at
different partition bases lock up the PE; DVE ops can't read two PSUM
operands; stride-0 (broadcast) DMA APs silently read garbage.

Host side: the wall clock is dominated by the ~45 MB/s, ~140 ms/array axon
tunnel, so ALL per-core inputs are packed into one uint8 blob (bf16 for
everything that only feeds matmuls, f32 elsewhere), the output returns as
bf16, output zero-buffers are created on-device, and the jitted shard_map
executable is cached across calls (the stock run_bass_kernel_spmd path
rebuilds and recompiles the jit every call).
"""
import numpy as np
import ml_dtypes
try:
    import jax as _jax
    _jax.config.update("jax_compilation_cache_dir", "/root/.jax_xla_cache")
    _jax.config.update("jax_persistent_cache_min_entry_size_bytes", 0)
    _jax.config.update("jax_persistent_cache_min_compile_time_secs", 0.0)
except Exception:
    pass
import concourse.bass as bass
import concourse.bacc as bacc
import concourse.mybir as mybir
import concourse.tile as tile

B, N, K, HID = 4, 8192, 32, 64
P = 128
NB1 = 64            # phase-1 blocks (full batch, 128 points each)
NB2 = 32            # phase-2 blocks (own half)
F32 = mybir.dt.float32
BF16 = mybir.dt.bfloat16
I16 = mybir.dt.int16
U8 = mybir.dt.uint8
AX = mybir.AxisListType
ALU = mybir.AluOpType
ACTF = mybir.ActivationFunctionType
H2 = N // 2
NPBF = ml_dtypes.bfloat16
NPF8 = ml_dtypes.float8_e4m3
F8 = mybir.dt.float8e4

# ---- packed input blob layout (per core) --------------------------------
_SECS = [
    # name, rows, cols, np dtype, bir dtype
    ("ft",   P, N, NPBF, BF16),
    ("efp",  3, N * K, NPF8, F8),
    ("idx1", 16, NB1 * 256, np.int16, I16),
    ("w1zr", P, P, NPBF, BF16),
    ("b1zr", P, 1, np.float32, F32),
    ("w1p",  3, P, NPF8, F8),
    ("w1qh", HID, HID, np.float32, F32),
    ("w1qx", HID, HID, NPBF, BF16),
    ("b1q",  HID, 1, np.float32, F32),
    ("w1pq", 3, HID, NPF8, F8),
    ("wl",   HID, 6 * HID, np.float32, F32),
    ("bl",   HID, 6, np.float32, F32),
]
_OFF = {}
_o = 0
for _n, _r, _c, _npdt, _ in _SECS:
    _OFF[_n] = _o
    _o += (_r * _c * np.dtype(_npdt).itemsize + 511) // 512 * 512
BLOB = _o

_cache = {}


def _host_view(row, name):
    r, c, npdt = next((s[1], s[2], s[3]) for s in _SECS if s[0] == name)
    nb = r * c * np.dtype(npdt).itemsize
    return row[_OFF[name]:_OFF[name] + nb].view(npdt).reshape(r, c)


def _build():
    nc = bacc.Bacc("TRN2", target_bir_lowering=False, debug=False)
    blobd = nc.dram_tensor("blob", [1, BLOB], U8, kind="ExternalInput").ap()

    def dview(name):
        r, c, npdt, bdt = next((s[1], s[2], s[3], s[4]) for s in _SECS if s[0] == name)
        nb = r * c * np.dtype(npdt).itemsize
        return blobd[:, _OFF[name]:_OFF[name] + nb].bitcast(bdt).rearrange(
            "a (p f) -> (a p) f", p=r)

    out_d = nc.dram_tensor("outT", [HID, H2], BF16, kind="ExternalOutput").ap()

    with tile.TileContext(nc) as tc:
        with tc.sbuf_pool(name="sb", bufs=1) as sb, \
             tc.sbuf_pool(name="gp", bufs=2) as gp, \
             tc.sbuf_pool(name="efs", bufs=2) as efs, \
             tc.sbuf_pool(name="y2p", bufs=3) as y2p, \
             tc.psum_pool(name="ps", bufs=4) as ps, \
             tc.psum_pool(name="pl", bufs=2) as pl:
            ft = sb.tile([P, N], BF16)
            tab = sb.tile([P, N], F32)
            y1T = sb.tile([P, N], F32)
            ixall = sb.tile([P, NB1 * 256], I16)
            w1zr = sb.tile([P, P], BF16)
            b1zr = sb.tile([P, 1], F32)
            w1p = sb.tile([3, P], F8)
            w1qh = sb.tile([P, HID], F32)    # rows 64:128 hold W1[2] rh rows
            w1qx = sb.tile([HID, HID], BF16)
            b1q = sb.tile([HID, 1], F32)
            w1pq = sb.tile([3, HID], F8)
            wl = sb.tile([P, 6 * HID], F32)
            bl = sb.tile([P, 6], F32)
            outb = sb.tile([P, H2], BF16)
            for t, name in ((ft, "ft"), (w1zr, "w1zr"), (b1zr, "b1zr"),
                            (w1p, "w1p"), (w1qx, "w1qx"), (b1q, "b1q"),
                            (w1pq, "w1pq")):
                nc.sync.dma_start(t[:], dview(name))
            nc.sync.dma_start(w1qh[HID:P, :], dview("w1qh"))
            for half in (0, HID):
                nc.sync.dma_start(wl[half:half + HID, :], dview("wl"))
                nc.sync.dma_start(bl[half:half + HID, :], dview("bl"))
            ixv = dview("idx1")
            for g in range(8):
                nc.sync.dma_start(ixall[g * 16:(g + 1) * 16, :], ixv)
            efd = dview("efp")

            # ---- z|r token table: tab = W1zr^T @ ft + b1zr --------------
            for g in range(16):
                c0 = g * 512
                pm = ps.tile([P, 512], F32, tag="ps")
                nc.tensor.matmul(out=pm[:], lhsT=w1zr[:], rhs=ft[:, c0:c0 + 512],
                                 start=True, stop=True)
                nc.scalar.activation(tab[:, c0:c0 + 512], pm[:], ACTF.Identity,
                                     bias=b1zr[:])

            def do_blocks(nblk, ch, wp_t, tab_ap, ydst_base):
                # gather + edge-add + max-pool; pooled cols land in y1T rows
                # 0:ch at columns ydst_base + blk*128
                for blk in range(nblk):
                    eft = efs.tile([3, K * P], F8, tag="ef")
                    nc.sync.dma_start(eft[:], efd[:, blk * K * P:(blk + 1) * K * P])
                    G = gp.tile([P, K * P], F32, tag="G")
                    nc.gpsimd.ap_gather(
                        out_ap=G[0:ch, :].rearrange("c (n d) -> c n d", d=1),
                        in_ap=tab_ap.rearrange("c (n d) -> c n d", d=1),
                        idxs_ap=ixall[0:ch, blk * 256:(blk + 1) * 256],
                        channels=ch, num_elems=N, d=1, num_idxs=K * P)
                    for s in range(8):
                        c0 = s * 512
                        pm = ps.tile([P, 512], F32, tag="ps")
                        nc.tensor.matmul(out=pm[0:ch, :], lhsT=wp_t,
                                         rhs=eft[:, c0:c0 + 512],
                                         start=True, stop=True)
                        nc.vector.tensor_tensor(
                            out=G[0:ch, c0:c0 + 512], in0=G[0:ch, c0:c0 + 512],
                            in1=pm[0:ch, :], op=ALU.add)
                    nc.vector.tensor_reduce(
                        out=y1T[0:ch, ydst_base + blk * P:ydst_base + (blk + 1) * P],
                        in_=G[0:ch, :].rearrange("c (p k) -> c p k", k=K),
                        axis=AX.X, op=ALU.max)

            def leaky(t_ap, parts, cols_per_chunk=4096):
                total = t_ap.shape[1]
                for c0 in range(0, total, cols_per_chunk):
                    cw = min(cols_per_chunk, total - c0)
                    sc = gp.tile([P, K * P], F32, tag="G")
                    nc.vector.tensor_scalar_mul(sc[0:parts, 0:cw],
                                                t_ap[:, c0:c0 + cw], 0.1)
                    nc.vector.tensor_tensor(out=t_ap[:, c0:c0 + cw],
                                            in0=t_ap[:, c0:c0 + cw],
                                            in1=sc[0:parts, 0:cw], op=ALU.max)

            def layers(src_lo, src_base, cols, w2c, b2c, w3c, b3c, act, dst_base):
                # src rows: y1T[0:64] if src_lo else y1T[64:128]; 512-col chunks
                # starting at src_base; dst = y1T[64:128, dst_base + ...]
                for g in range(cols // 512):
                    c0 = g * 512
                    src = (y1T[0:HID, src_base + c0:src_base + c0 + 512] if src_lo
                           else y1T[HID:P, src_base + c0:src_base + c0 + 512])
                    wbase = 0 if src_lo else HID
                    pm = pl.tile([P, 512], F32, tag="pl")
                    nc.tensor.matmul(out=pm[0:HID, :],
                                     lhsT=wl[wbase:wbase + HID, w2c:w2c + HID],
                                     rhs=src, start=True, stop=True)
                    y2 = y2p.tile([P, 512], F32, tag="y2")
                    nc.scalar.activation(y2[0:HID, :], pm[0:HID, :], ACTF.Identity,
                                         bias=bl[0:HID, b2c:b2c + 1])
                    sc2 = y2p.tile([P, 512], F32, tag="y2")
                    nc.vector.tensor_scalar_mul(sc2[0:HID, :], y2[0:HID, :], 0.1)
                    nc.vector.tensor_tensor(out=y2[0:HID, :], in0=y2[0:HID, :],
                                            in1=sc2[0:HID, :], op=ALU.max)
                    pm2 = pl.tile([P, 512], F32, tag="pl")
                    nc.tensor.matmul(out=pm2[HID:P, :],
                                     lhsT=wl[0:HID, w3c:w3c + HID],
                                     rhs=y2[0:HID, :], start=True, stop=True)
                    nc.scalar.activation(
                        y1T[HID:P, dst_base + c0:dst_base + c0 + 512],
                        pm2[HID:P, :], act, bias=bl[HID:P, b3c:b3c + 1])

            # ---- phase 1: gather+pool z|r for all 64 blocks -------------
            do_blocks(NB1, P, w1p[:], tab[:], 0)
            leaky(y1T[:], P)
            # ---- r layers (all tokens), in place over r rows ------------
            layers(False, 0, N, HID, 1, 4 * HID, 4, ACTF.Sigmoid, 0)
            # ---- rh = r * h ---------------------------------------------
            for c0 in (0, H2):
                nc.vector.tensor_tensor(out=y1T[HID:P, c0:c0 + H2],
                                        in0=y1T[HID:P, c0:c0 + H2],
                                        in1=ft[HID:P, c0:c0 + H2], op=ALU.mult)
            # ---- q token table into tab rows 0:64 -----------------------
            # (accumulating matmuls with different lhsT partition bases lock
            # up the PE on HW, and DVE can't read two PSUM operands, so:
            # matmul -> ACT copy to SBUF, matmul -> DVE add -> ACT bias)
            for g in range(16):
                c0 = g * 512
                pm = pl.tile([P, 512], F32, tag="pl")
                nc.tensor.matmul(out=pm[0:HID, :], lhsT=w1qh[HID:P, :],
                                 rhs=y1T[HID:P, c0:c0 + 512], start=True, stop=True)
                t = y2p.tile([P, 512], F32, tag="y2")
                nc.scalar.activation(t[0:HID, :], pm[0:HID, :], ACTF.Identity)
                pmb = pl.tile([P, 512], F32, tag="pl")
                nc.tensor.matmul(out=pmb[0:HID, :], lhsT=w1qx[:],
                                 rhs=ft[0:HID, c0:c0 + 512], start=True, stop=True)
                nc.vector.tensor_tensor(out=t[0:HID, :], in0=t[0:HID, :],
                                        in1=pmb[0:HID, :], op=ALU.add)
                nc.scalar.activation(tab[0:HID, c0:c0 + 512], t[0:HID, :],
                                     ACTF.Identity, bias=b1q[:])
            # ---- z layers (own half) -> zg in y1T[64:128, 0:4096] -------
            layers(True, 0, H2, 0, 0, 3 * HID, 3, ACTF.Sigmoid, 0)
            # ---- phase 2: gather+pool q (own half); pooled q lands in
            #      y1T[0:64, 4096:8192] (dead other-half z region) --------
            do_blocks(NB2, HID, w1pq[:], tab[0:HID, :], H2)
            leaky(y1T[0:HID, H2:N], HID)
            # ---- q layers -> qg in y1T[64:128, 4096:8192] ---------------
            layers(True, H2, H2, 2 * HID, 2, 5 * HID, 5, ACTF.Tanh, H2)
            # ---- out = h + z*(q - h) ------------------------------------
            qg = y1T[HID:P, H2:N]
            nc.vector.tensor_tensor(out=qg, in0=qg, in1=ft[HID:P, 0:H2],
                                    op=ALU.subtract)
            nc.vector.tensor_tensor(out=qg, in0=qg, in1=y1T[HID:P, 0:H2],
                                    op=ALU.mult)
            nc.vector.tensor_tensor(out=outb[HID:P, :], in0=qg,
                                    in1=ft[HID:P, 0:H2], op=ALU.add)
            nc.sync.dma_start(out_d, outb[HID:P, :])
    nc.compile()
    return nc


def _fill_weights(row, inputs):
    W1 = np.asarray(inputs["W1"], np.float32)
    b1 = np.asarray(inputs["b1"], np.float32)
    W2 = np.asarray(inputs["W2"], np.float32)
    b2 = np.asarray(inputs["b2"], np.float32)
    W3 = np.asarray(inputs["W3"], np.float32)
    b3 = np.asarray(inputs["b3"], np.float32)
    _host_view(row, "w1zr")[:] = np.concatenate([
        np.concatenate([W1[0][HID:P], W1[1][HID:P]], 1),
        np.concatenate([W1[0][0:HID], W1[1][0:HID]], 1)], 0).astype(NPBF)
    _host_view(row, "b1zr")[:] = np.concatenate([b1[0], b1[1]])[:, None]
    _host_view(row, "w1p")[:] = np.concatenate([W1[0][P:], W1[1][P:]], 1).astype(NPF8)
    _host_view(row, "w1qh")[:] = W1[2][0:HID]
    _host_view(row, "w1qx")[:] = W1[2][HID:P].astype(NPBF)
    _host_view(row, "b1q")[:] = b1[2][:, None]
    _host_view(row, "w1pq")[:] = W1[2][P:].astype(NPF8)
    _host_view(row, "wl")[:] = np.concatenate(
        [W2[0], W2[1], W2[2], W3[0], W3[1], W3[2]], 1)
    _host_view(row, "bl")[:] = np.stack(
        [b2[0], b2[1], b2[2], b3[0], b3[1], b3[2]], 1)


def _fill_core(row, inputs, b, hh):
    x = np.asarray(inputs["x"])
    h = np.asarray(inputs["h"])
    nid = np.asarray(inputs["neigh_idx"])
    ef = np.asarray(inputs["edge_feats"])
    rot = (np.arange(N) + hh * H2) % N        # local token -> global
    ftv = _host_view(row, "ft")
    ftv[0:HID] = x[b][rot].T.astype(NPBF)
    ftv[HID:P] = h[b][rot].T.astype(NPBF)
    _host_view(row, "efp")[:] = ef[b][rot].transpose(2, 0, 1).reshape(3, N * K).astype(NPF8)
    loc = ((nid[b][rot] - hh * H2) % N).astype(np.int16)     # [N, K] local
    _host_view(row, "idx1")[:] = loc.reshape(NB1, 256, 16).transpose(2, 0, 1).reshape(16, NB1 * 256)


def _make_runner(nc, n_cores):
    """Build the jitted shard_map executable ONCE and reuse across calls.

    Mirrors bass2jax.run_bass_via_pjrt, which rebuilds (and thus retraces +
    recompiles) the jit on every invocation.  Output zero-buffers are
    created on-device instead of being shipped through the tunnel.
    """
    import jax
    import jax.numpy as jnp
    from jax.sharding import Mesh, PartitionSpec
    from jax.experimental.shard_map import shard_map
    from concourse import bass2jax

    bass2jax.install_neuronx_cc_hook()
    assert nc.dbg_addr is None
    pname = nc.partition_id_tensor.name if nc.partition_id_tensor else None

    in_names, out_names, out_avals = [], [], []
    for alloc in nc.m.functions[0].allocations:
        if not isinstance(alloc, mybir.MemoryLocationSet):
            continue
        name = alloc.memorylocations[0].name
        if alloc.kind == "ExternalInput":
            if name != pname:
                in_names.append(name)
        elif alloc.kind == "ExternalOutput":
            out_names.append(name)
            out_avals.append(jax.core.ShapedArray(
                tuple(alloc.tensor_shape), mybir.dt.np(alloc.dtype)))
    assert in_names == ["blob"], in_names
    all_names = in_names + out_names + ([pname] if pname else [])

    def _body(*args):
        operands = list(args)
        if pname is not None:
            operands.append(bass2jax.partition_id_tensor())
        outs = bass2jax._bass_exec_p.bind(
            *operands,
            out_avals=tuple(out_avals),
            in_names=tuple(all_names),
            out_names=tuple(out_names),
            lowering_input_output_aliases=(),
            sim_require_finite=True,
            sim_require_nnan=True,
            nc=nc,
        )
        return tuple(outs)

    devices = jax.devices()[:n_cores]
    mesh = Mesh(np.asarray(devices), ("core",))
    nouts = len(out_names)
    sharded = jax.jit(
        shard_map(_body, mesh=mesh,
                  in_specs=(PartitionSpec("core"),) * (1 + nouts),
                  out_specs=(PartitionSpec("core"),) * nouts,
                  check_rep=False),
        keep_unused=True)
    # Output zero-buffers: the custom call needs them as PARAMETERS (the
    # neuronx hook rejects constant operands), but their content never
    # changes and the kernel overwrites every output element, so commit
    # them to the devices once and reuse across calls.
    sh = jax.sharding.NamedSharding(mesh, PartitionSpec("core"))
    zeros_dev = [jax.device_put(
        np.zeros((n_cores * a.shape[0], *a.shape[1:]), a.dtype), sh)
        for a in out_avals]

    def run(big):
        # big: [n_cores, BLOB] uint8 -> shard_map global [(n_cores*1), BLOB]
        out_arrs = sharded(big.reshape(n_cores * 1, BLOB), *zeros_dev)
        return [np.asarray(a) for a in out_arrs]

    return run


def kernel(**inputs):
    if "nc" not in _cache:
        _cache["nc"] = _build()
    nc = _cache["nc"]
    from concurrent.futures import ThreadPoolExecutor
    big = np.empty((8, BLOB), np.uint8)
    _fill_weights(big[0], inputs)
    woff = _OFF["w1zr"]
    for c in range(1, 8):
        big[c, woff:] = big[0, woff:]
    with ThreadPoolExecutor(8) as ex:
        list(ex.map(lambda c: _fill_core(big[c], inputs, c // 2, c % 2), range(8)))
    out = np.empty((B, N, HID), np.float32)
    try:
        if "run" not in _cache:
            _cache["run"] = _make_runner(nc, 8)
        outT = _cache["run"](big)[0]           # [8*64, 4096] bf16
        outT = outT.reshape(8, HID, H2)
        for c in range(8):
            b, hh = c // 2, c % 2
            out[b, hh * H2:(hh + 1) * H2] = outT[c].T.astype(np.float32)
    except Exception:
        import concourse.bass_utils as bass_utils
        in_maps = [{"blob": big[c:c + 1]} for c in range(8)]
        res = bass_utils.run_bass_kernel_spmd(nc, in_maps, core_ids=list(range(8)))
        for c in range(8):
            b, hh = c // 2, c % 2
            out[b, hh * H2:(hh + 1) * H2] = res.results[c]["outT"].T.astype(np.float32)
    return out


# revision 12
# speedup vs baseline: 1.0298x; 1.0298x over previous
"""GRU/SetConv GNN message-passing kernel for 8 TRN2 NeuronCores.

Strategy (data-parallel per the sharding hint):
  core c -> batch b=c//2, point-half hh=c%2; tokens rotated so the core's own
  half is local tokens 0..4095.  Everything runs in transposed layout
  [channels, tokens]:
    1. per-token tables U_zr^T = W1zr^T @ [x;h]^T + b (PE, 16 chunks)
    2. per 128-point block: gpsimd ap_gather pulls 4096 neighbor columns
       of the SBUF-resident table (no DMA descriptors), PE adds the edge
       term ef @ W1[128:131] via 3-deep matmuls into PSUM, DVE adds and
       max-pools over k=32 with a single strided reduce.
    3. tiny MLPs run transposed with per-partition biases fused into ACT
       (Identity/Sigmoid/Tanh); LReLU via DVE mul+max.
  Gate r is computed for the full batch on both half-cores (cheaper than a
  cross-core exchange); z and q only for the own half.  The q-table reuses
  the z-table rows; z/q gate outputs and the final GRU combine reuse dead
  regions of the pooled buffer, so SBUF holds everything without spills.

HW quirks found the hard way: accumulating matmuls whose lhsT live at
different partition bases lock up the PE; DVE ops can't read two PSUM
operands; stride-0 (broadcast) DMA APs silently read garbage.

Host side: the wall clock is dominated by the ~45 MB/s, ~140 ms/array axon
tunnel, so ALL per-core inputs are packed into one uint8 blob (bf16 for
everything that only feeds matmuls, f32 elsewhere), the output returns as
bf16, output zero-buffers are created on-device, and the jitted shard_map
executable is cached across calls (the stock run_bass_kernel_spmd path
rebuilds and recompiles the jit every call).
"""
import numpy as np
import ml_dtypes
try:
    import jax as _jax
    _jax.config.update("jax_compilation_cache_dir", "/root/.jax_xla_cache")
    _jax.config.update("jax_persistent_cache_min_entry_size_bytes", 0)
    _jax.config.update("jax_persistent_cache_min_compile_time_secs", 0.0)
except Exception:
    pass
import concourse.bass as bass
import concourse.bacc as bacc
import concourse.mybir as mybir
import concourse.tile as tile

B, N, K, HID = 4, 8192, 32, 64
P = 128
NB1 = 64            # phase-1 blocks (full batch, 128 points each)
NB2 = 32            # phase-2 blocks (own half)
F32 = mybir.dt.float32
BF16 = mybir.dt.bfloat16
I16 = mybir.dt.int16
U8 = mybir.dt.uint8
AX = mybir.AxisListType
ALU = mybir.AluOpType
ACTF = mybir.ActivationFunctionType
H2 = N // 2
NPBF = ml_dtypes.bfloat16
NPF8 = ml_dtypes.float8_e4m3
F8 = mybir.dt.float8e4

# ---- packed input blob layout (per core) --------------------------------
_SECS = [
    # name, rows, cols, np dtype, bir dtype
    ("ft",   P, N, NPBF, BF16),
    ("efp",  3, N * K, NPF8, F8),
    ("idx1", 16, NB1 * 256, np.int16, I16),
    ("w1zr", P, P, NPBF, BF16),
    ("b1zr", P, 1, np.float32, F32),
    ("w1p",  3, P, NPF8, F8),
    ("w1qh", HID, HID, np.float32, F32),
    ("w1qx", HID, HID, NPBF, BF16),
    ("b1q",  HID, 1, np.float32, F32),
    ("w1pq", 3, HID, NPF8, F8),
    ("wl",   HID, 6 * HID, np.float32, F32),
    ("bl",   HID, 6, np.float32, F32),
]
_OFF = {}
_o = 0
for _n, _r, _c, _npdt, _ in _SECS:
    _OFF[_n] = _o
    _o += (_r * _c * np.dtype(_npdt).itemsize + 511) // 512 * 512
BLOB = _o

_cache = {}


def _host_view(row, name):
    r, c, npdt = next((s[1], s[2], s[3]) for s in _SECS if s[0] == name)
    nb = r * c * np.dtype(npdt).itemsize
    return row[_OFF[name]:_OFF[name] + nb].view(npdt).reshape(r, c)


def _build():
    nc = bacc.Bacc("TRN2", target_bir_lowering=False, debug=False)
    blobd = nc.dram_tensor("blob", [1, BLOB], U8, kind="ExternalInput").ap()

    def dview(name):
        r, c, npdt, bdt = next((s[1], s[2], s[3], s[4]) for s in _SECS if s[0] == name)
        nb = r * c * np.dtype(npdt).itemsize
        return blobd[:, _OFF[name]:_OFF[name] + nb].bitcast(bdt).rearrange(
            "a (p f) -> (a p) f", p=r)

    out_d = nc.dram_tensor("outT", [HID, H2], BF16, kind="ExternalOutput").ap()

    with tile.TileContext(nc) as tc:
        with tc.sbuf_pool(name="sb", bufs=1) as sb, \
             tc.sbuf_pool(name="gp", bufs=2) as gp, \
             tc.sbuf_pool(name="efs", bufs=2) as efs, \
             tc.sbuf_pool(name="y2p", bufs=3) as y2p, \
             tc.psum_pool(name="ps", bufs=4) as ps, \
             tc.psum_pool(name="pl", bufs=2) as pl:
            ft = sb.tile([P, N], BF16)
            tab = sb.tile([P, N], F32)
            y1T = sb.tile([P, N], F32)
            ixall = sb.tile([P, NB1 * 256], I16)
            w1zr = sb.tile([P, P], BF16)
            b1zr = sb.tile([P, 1], F32)
            w1p = sb.tile([3, P], F8)
            w1qh = sb.tile([P, HID], F32)    # rows 64:128 hold W1[2] rh rows
            w1qx = sb.tile([HID, HID], BF16)
            b1q = sb.tile([HID, 1], F32)
            w1pq = sb.tile([3, HID], F8)
            wl = sb.tile([P, 6 * HID], F32)
            bl = sb.tile([P, 6], F32)
            outb = sb.tile([P, H2], BF16)
            for t, name in ((ft, "ft"), (w1zr, "w1zr"), (b1zr, "b1zr"),
                            (w1p, "w1p"), (w1qx, "w1qx"), (b1q, "b1q"),
                            (w1pq, "w1pq")):
                nc.sync.dma_start(t[:], dview(name))
            nc.sync.dma_start(w1qh[HID:P, :], dview("w1qh"))
            for half in (0, HID):
                nc.sync.dma_start(wl[half:half + HID, :], dview("wl"))
                nc.sync.dma_start(bl[half:half + HID, :], dview("bl"))
            ixv = dview("idx1")
            for g in range(8):
                nc.sync.dma_start(ixall[g * 16:(g + 1) * 16, :], ixv)
            efd = dview("efp")

            # ---- z|r token table: tab = W1zr^T @ ft + b1zr --------------
            for g in range(16):
                c0 = g * 512
                pm = ps.tile([P, 512], F32, tag="ps")
                nc.tensor.matmul(out=pm[:], lhsT=w1zr[:], rhs=ft[:, c0:c0 + 512],
                                 start=True, stop=True)
                nc.scalar.activation(tab[:, c0:c0 + 512], pm[:], ACTF.Identity,
                                     bias=b1zr[:])

            def do_blocks(nblk, ch, wp_t, tab_ap, ydst_base):
                # gather + edge-add + max-pool; pooled cols land in y1T rows
                # 0:ch at columns ydst_base + blk*128
                for blk in range(nblk):
                    eft = efs.tile([3, K * P], F8, tag="ef")
                    nc.sync.dma_start(eft[:], efd[:, blk * K * P:(blk + 1) * K * P])
                    G = gp.tile([P, K * P], F32, tag="G")
                    nc.gpsimd.ap_gather(
                        out_ap=G[0:ch, :].rearrange("c (n d) -> c n d", d=1),
                        in_ap=tab_ap.rearrange("c (n d) -> c n d", d=1),
                        idxs_ap=ixall[0:ch, blk * 256:(blk + 1) * 256],
                        channels=ch, num_elems=N, d=1, num_idxs=K * P)
                    for s in range(8):
                        c0 = s * 512
                        pm = ps.tile([P, 512], F32, tag="ps")
                        nc.tensor.matmul(out=pm[0:ch, :], lhsT=wp_t,
                                         rhs=eft[:, c0:c0 + 512],
                                         start=True, stop=True)
                        nc.vector.tensor_tensor(
                            out=G[0:ch, c0:c0 + 512], in0=G[0:ch, c0:c0 + 512],
                            in1=pm[0:ch, :], op=ALU.add)
                    nc.vector.tensor_reduce(
                        out=y1T[0:ch, ydst_base + blk * P:ydst_base + (blk + 1) * P],
                        in_=G[0:ch, :].rearrange("c (p k) -> c p k", k=K),
                        axis=AX.X, op=ALU.max)

            def leaky(t_ap, parts, cols_per_chunk=4096):
                total = t_ap.shape[1]
                for c0 in range(0, total, cols_per_chunk):
                    cw = min(cols_per_chunk, total - c0)
                    sc = gp.tile([P, K * P], F32, tag="G")
                    nc.vector.tensor_scalar_mul(sc[0:parts, 0:cw],
                                                t_ap[:, c0:c0 + cw], 0.1)
                    nc.vector.tensor_tensor(out=t_ap[:, c0:c0 + cw],
                                            in0=t_ap[:, c0:c0 + cw],
                                            in1=sc[0:parts, 0:cw], op=ALU.max)

            def layers(src_lo, src_base, cols, w2c, b2c, w3c, b3c, act, dst_base):
                # src rows: y1T[0:64] if src_lo else y1T[64:128]; 512-col chunks
                # starting at src_base; dst = y1T[64:128, dst_base + ...]
                for g in range(cols // 512):
                    c0 = g * 512
                    src = (y1T[0:HID, src_base + c0:src_base + c0 + 512] if src_lo
                           else y1T[HID:P, src_base + c0:src_base + c0 + 512])
                    wbase = 0 if src_lo else HID
                    pm = pl.tile([P, 512], F32, tag="pl")
                    nc.tensor.matmul(out=pm[0:HID, :],
                                     lhsT=wl[wbase:wbase + HID, w2c:w2c + HID],
                                     rhs=src, start=True, stop=True)
                    y2 = y2p.tile([P, 512], F32, tag="y2")
                    nc.scalar.activation(y2[0:HID, :], pm[0:HID, :], ACTF.Identity,
                                         bias=bl[0:HID, b2c:b2c + 1])
                    sc2 = y2p.tile([P, 512], F32, tag="y2")
                    nc.vector.tensor_scalar_mul(sc2[0:HID, :], y2[0:HID, :], 0.1)
                    nc.vector.tensor_tensor(out=y2[0:HID, :], in0=y2[0:HID, :],
                                            in1=sc2[0:HID, :], op=ALU.max)
                    pm2 = pl.tile([P, 512], F32, tag="pl")
                    nc.tensor.matmul(out=pm2[HID:P, :],
                                     lhsT=wl[0:HID, w3c:w3c + HID],
                                     rhs=y2[0:HID, :], start=True, stop=True)
                    nc.scalar.activation(
                        y1T[HID:P, dst_base + c0:dst_base + c0 + 512],
                        pm2[HID:P, :], act, bias=bl[HID:P, b3c:b3c + 1])

            # ---- phase 1: gather+pool z|r for all 64 blocks -------------
            do_blocks(NB1, P, w1p[:], tab[:], 0)
            leaky(y1T[:], P)
            # ---- r layers (all tokens), in place over r rows ------------
            layers(False, 0, N, HID, 1, 4 * HID, 4, ACTF.Sigmoid, 0)
            # ---- rh = r * h ---------------------------------------------
            for c0 in (0, H2):
                nc.vector.tensor_tensor(out=y1T[HID:P, c0:c0 + H2],
                                        in0=y1T[HID:P, c0:c0 + H2],
                                        in1=ft[HID:P, c0:c0 + H2], op=ALU.mult)
            # ---- q token table into tab rows 0:64 -----------------------
            # (accumulating matmuls with different lhsT partition bases lock
            # up the PE on HW, and DVE can't read two PSUM operands, so:
            # matmul -> ACT copy to SBUF, matmul -> DVE add -> ACT bias)
            for g in range(16):
                c0 = g * 512
                pm = pl.tile([P, 512], F32, tag="pl")
                nc.tensor.matmul(out=pm[0:HID, :], lhsT=w1qh[HID:P, :],
                                 rhs=y1T[HID:P, c0:c0 + 512], start=True, stop=True)
                t = y2p.tile([P, 512], F32, tag="y2")
                nc.scalar.activation(t[0:HID, :], pm[0:HID, :], ACTF.Identity)
                pmb = pl.tile([P, 512], F32, tag="pl")
                nc.tensor.matmul(out=pmb[0:HID, :], lhsT=w1qx[:],
                                 rhs=ft[0:HID, c0:c0 + 512], start=True, stop=True)
                nc.vector.tensor_tensor(out=t[0:HID, :], in0=t[0:HID, :],
                                        in1=pmb[0:HID, :], op=ALU.add)
                nc.scalar.activation(tab[0:HID, c0:c0 + 512], t[0:HID, :],
                                     ACTF.Identity, bias=b1q[:])
            # ---- z layers (own half) -> zg in y1T[64:128, 0:4096] -------
            layers(True, 0, H2, 0, 0, 3 * HID, 3, ACTF.Sigmoid, 0)
            # ---- phase 2: gather+pool q (own half); pooled q lands in
            #      y1T[0:64, 4096:8192] (dead other-half z region) --------
            do_blocks(NB2, HID, w1pq[:], tab[0:HID, :], H2)
            leaky(y1T[0:HID, H2:N], HID)
            # ---- q layers -> qg in y1T[64:128, 4096:8192] ---------------
            layers(True, H2, H2, 2 * HID, 2, 5 * HID, 5, ACTF.Tanh, H2)
            # ---- out = h + z*(q - h) ------------------------------------
            qg = y1T[HID:P, H2:N]
            nc.vector.tensor_tensor(out=qg, in0=qg, in1=ft[HID:P, 0:H2],
                                    op=ALU.subtract)
            nc.vector.tensor_tensor(out=qg, in0=qg, in1=y1T[HID:P, 0:H2],
                                    op=ALU.mult)
            nc.vector.tensor_tensor(out=outb[HID:P, :], in0=qg,
                                    in1=ft[HID:P, 0:H2], op=ALU.add)
            nc.sync.dma_start(out_d, outb[HID:P, :])
    nc.compile()
    return nc


def _fill_weights(row, inputs):
    W1 = np.asarray(inputs["W1"], np.float32)
    b1 = np.asarray(inputs["b1"], np.float32)
    W2 = np.asarray(inputs["W2"], np.float32)
    b2 = np.asarray(inputs["b2"], np.float32)
    W3 = np.asarray(inputs["W3"], np.float32)
    b3 = np.asarray(inputs["b3"], np.float32)
    _host_view(row, "w1zr")[:] = np.concatenate([
        np.concatenate([W1[0][HID:P], W1[1][HID:P]], 1),
        np.concatenate([W1[0][0:HID], W1[1][0:HID]], 1)], 0).astype(NPBF)
    _host_view(row, "b1zr")[:] = np.concatenate([b1[0], b1[1]])[:, None]
    _host_view(row, "w1p")[:] = np.concatenate([W1[0][P:], W1[1][P:]], 1).astype(NPF8)
    _host_view(row, "w1qh")[:] = W1[2][0:HID]
    _host_view(row, "w1qx")[:] = W1[2][HID:P].astype(NPBF)
    _host_view(row, "b1q")[:] = b1[2][:, None]
    _host_view(row, "w1pq")[:] = W1[2][P:].astype(NPF8)
    _host_view(row, "wl")[:] = np.concatenate(
        [W2[0], W2[1], W2[2], W3[0], W3[1], W3[2]], 1)
    _host_view(row, "bl")[:] = np.stack(
        [b2[0], b2[1], b2[2], b3[0], b3[1], b3[2]], 1)


def _fill_core(row, inputs, b, hh):
    x = np.asarray(inputs["x"])
    h = np.asarray(inputs["h"])
    nid = np.asarray(inputs["neigh_idx"])
    ef = np.asarray(inputs["edge_feats"])
    rot = (np.arange(N) + hh * H2) % N        # local token -> global
    ftv = _host_view(row, "ft")
    ftv[0:HID] = x[b][rot].T.astype(NPBF)
    ftv[HID:P] = h[b][rot].T.astype(NPBF)
    _host_view(row, "efp")[:] = ef[b][rot].transpose(2, 0, 1).reshape(3, N * K).astype(NPF8)
    loc = ((nid[b][rot] - hh * H2) % N).astype(np.int16)     # [N, K] local
    _host_view(row, "idx1")[:] = loc.reshape(NB1, 256, 16).transpose(2, 0, 1).reshape(16, NB1 * 256)


def _make_runner(nc, n_cores):
    """Build the jitted shard_map executable ONCE and reuse across calls.

    Mirrors bass2jax.run_bass_via_pjrt, which rebuilds (and thus retraces +
    recompiles) the jit on every invocation.  Output zero-buffers are
    created on-device instead of being shipped through the tunnel.
    """
    import jax
    import jax.numpy as jnp
    from jax.sharding import Mesh, PartitionSpec
    from jax.experimental.shard_map import shard_map
    from concourse import bass2jax

    bass2jax.install_neuronx_cc_hook()
    assert nc.dbg_addr is None
    pname = nc.partition_id_tensor.name if nc.partition_id_tensor else None

    in_names, out_names, out_avals = [], [], []
    for alloc in nc.m.functions[0].allocations:
        if not isinstance(alloc, mybir.MemoryLocationSet):
            continue
        name = alloc.memorylocations[0].name
        if alloc.kind == "ExternalInput":
            if name != pname:
                in_names.append(name)
        elif alloc.kind == "ExternalOutput":
            out_names.append(name)
            out_avals.append(jax.core.ShapedArray(
                tuple(alloc.tensor_shape), mybir.dt.np(alloc.dtype)))
    assert in_names == ["blob"], in_names
    all_names = in_names + out_names + ([pname] if pname else [])

    def _body(*args):
        operands = list(args)
        if pname is not None:
            operands.append(bass2jax.partition_id_tensor())
        outs = bass2jax._bass_exec_p.bind(
            *operands,
            out_avals=tuple(out_avals),
            in_names=tuple(all_names),
            out_names=tuple(out_names),
            lowering_input_output_aliases=(),
            sim_require_finite=True,
            sim_require_nnan=True,
            nc=nc,
        )
        return tuple(outs)

    devices = jax.devices()[:n_cores]
    mesh = Mesh(np.asarray(devices), ("core",))
    nouts = len(out_names)
    sharded = jax.jit(
        shard_map(_body, mesh=mesh,
                  in_specs=(PartitionSpec("core"),) * (1 + nouts),
                  out_specs=(PartitionSpec("core"),) * nouts,
                  check_rep=False),
        keep_unused=True)
    # Output zero-buffers: the custom call needs them as PARAMETERS (the
    # neuronx hook rejects constant operands), but their content never
    # changes and the kernel overwrites every output element, so commit
    # them to the devices once and reuse across calls.
    sh = jax.sharding.NamedSharding(mesh, PartitionSpec("core"))
    zeros_dev = [jax.device_put(
        np.zeros((n_cores * a.shape[0], *a.shape[1:]), a.dtype), sh)
        for a in out_avals]

    def run(big):
        # big: [n_cores, BLOB] uint8 -> shard_map global [(n_cores*1), BLOB]
        out_arrs = sharded(big.reshape(n_cores * 1, BLOB), *zeros_dev)
        return [np.asarray(a) for a in out_arrs]

    return run


def kernel(**inputs):
    if "nc" not in _cache:
        _cache["nc"] = _build()
    nc = _cache["nc"]
    from concurrent.futures import ThreadPoolExecutor
    big = np.empty((8, BLOB), np.uint8)
    _fill_weights(big[0], inputs)
    woff = _OFF["w1zr"]
    for c in range(1, 8):
        big[c, woff:] = big[0, woff:]
    with ThreadPoolExecutor(8) as ex:
        list(ex.map(lambda c: _fill_core(big[c], inputs, c // 2, c % 2), range(8)))
    out = np.empty((B, N, HID), np.float32)
    try:
        if "run" not in _cache:
            _cache["run"] = _make_runner(nc, 8)
        outT = _cache["run"](big)[0]           # [8*64, 4096] bf16
        outT = outT.reshape(8, HID, H2)
        for c in range(8):
            b, hh = c // 2, c % 2
            out[b, hh * H2:(hh + 1) * H2] = outT[c].T.astype(np.float32)
    except Exception:
        _cache.pop("run", None)
        try:
            import concourse.bass_utils as bass_utils
            in_maps = [{"blob": big[c:c + 1]} for c in range(8)]
            res = bass_utils.run_bass_kernel_spmd(nc, in_maps, core_ids=list(range(8)))
            for c in range(8):
                b, hh = c // 2, c % 2
                out[b, hh * H2:(hh + 1) * H2] = res.results[c]["outT"].T.astype(np.float32)
        except Exception:
            # last resort: simulate (slow but correct even with a wedged device)
            from concourse.bass_interp import CoreSim
            for c in range(8):
                b, hh = c // 2, c % 2
                sim = CoreSim(nc, trace=False)
                sim.mem_tensor("blob")[:] = big[c:c + 1]
                sim.event_loop()
                out[b, hh * H2:(hh + 1) * H2] = np.asarray(
                    sim.mem_tensor("outT")).T.astype(np.float32)
    return out
